# revision 1
# baseline (speedup 1.0000x reference)
"""Fused AttentionBlock (GroupNorm + single-head attention + proj + residual)
for Trainium2, Bass/Tile, data-parallel over batch across 8 NeuronCores.

Math (per sample, C=256 channels, N=1024 spatial):
  xn = GroupNorm(x) * gn_w + gn_b            (8 groups of 32 channels)
  q,k,v = qkv_w @ xn + qkv_b                 (1x1 conv == matmul over C)
  S^T[k,q] = sum_c k[c,k] q[c,q]             (computed transposed so softmax
                                              runs without any transposes)
  P = exp(S^T / sqrt(C))                     (no max-subtract; logits < 3)
  O_un[c,q] = sum_k v0[c,k] P[k,q]           (PSUM-accumulated over k tiles,
                                              v0 = v without bias)
  O = O_un * (1 / colsum(P))                 (deferred softmax denominator)
  out = proj_w @ O + proj_b' + x             (proj_b' = proj_b + proj_w@v_bias
                                              folded on host: softmax rows sum
                                              to 1, so v_bias passes through)

Key design points:
- q,k are never materialized: softmax over k is invariant to per-q-column
  constants, so S^T = xn^T (Wk^T Wq) xn with a single fused operand
  u = (Wk^T Wq) @ xn (25% fewer matmul FLOPs than the q/k/v decomposition);
  the surviving k-side logit bias rides as an extra column of the v matmul
  and feeds exp's per-partition bias port.
- All matmuls run in float32r (fp32 bits, 1 cycle/row on the PE vs 4 for
  strict fp32); producers write f32r-rounded outputs as the ISA requires.
- Channels live on SBUF partitions for x/xn/u/O; v is computed transposed
  (spatial on partitions) by swapping matmul operands, so the kernel
  contains no transposes at all.  The softmax denominator colsum(P) (a
  partition-axis reduction) is an all-ones matmul that also broadcasts the
  result across partitions for free, and normalization is applied
  post-projection (linearity) so the proj matmuls never wait on it.
- GroupNorm group stats cross partitions via tiny mask matmuls; rstd is
  computed as exp(-0.5*ln(var+eps)) so every ACT function lives in one
  activation-table set (no table reloads).
- Emission is software-pipelined by hand (next sample's load/stats/affine
  staged between the current sample's attention halves) and engine
  assignment of PSUM readouts / colsum partials is tuned against the
  concourse cost-model timeline; per-core makespan ~99.6 us with
  the PE (the bottleneck engine) ~83% busy.
"""

import os
from contextlib import ExitStack

import numpy as np

import concourse.bass as bass
import concourse.mybir as mybir
import concourse.tile as tile
from concourse.bass_utils import run_bass_kernel_spmd

# Problem shapes (hardcoded per spec nn_AttentionBlock_62397284876438)
B, C, HIMG, WIMG = 32, 256, 32, 32
HW = HIMG * WIMG          # 1024 spatial positions
G = 8                     # groupnorm groups
EPS = 1e-5
NCORES = 8
NS = B // NCORES          # samples per core = 4
P = 128                   # SBUF partitions
CT = C // P               # channel tiles = 2
NT = HW // P              # spatial tiles = 8
FD = 512                  # matmul moving free dim (one PSUM bank of fp32)
NH = HW // FD             # q halves = 2
SCALE = C ** -0.5
F32 = mybir.dt.float32
# float32r: fp32 bits, PE runs at 1 cycle/row (vs 4 for strict fp32)
MM_DT = mybir.dt.float32r

last_results = None       # BassKernelResults of the most recent run (for test.py)
_nc_cache = {}


def _hs(h):
    return slice(h * FD, (h + 1) * FD)


def _ms(m):
    return slice(m * P, (m + 1) * P)


def _build_nc():
    nc = bass.Bass()

    x_d = nc.dram_tensor("x", [NS, CT, P, HW], F32, kind="ExternalInput")
    WW = 3 * C + 4  # wu | wv+bias-col+pad | wp, concatenated along free
    wall_d = nc.dram_tensor("wall", [CT, P, WW], MM_DT, kind="ExternalInput")
    sm_d = nc.dram_tensor("sm", [P, 6 + CT * G], F32, kind="ExternalInput")
    bcmask_d = nc.dram_tensor("bcmask", [G, CT * P], F32, kind="ExternalInput")
    out_d = nc.dram_tensor("out", [NS, CT, P, HW], F32, kind="ExternalOutput")

    AL = mybir.AluOpType
    AF = mybir.ActivationFunctionType

    with tile.TileContext(nc) as tc, ExitStack() as ctx:
        consts = ctx.enter_context(tc.tile_pool(name="consts", bufs=1))
        xpool = ctx.enter_context(tc.tile_pool(name="xpool", bufs=4))
        xnpool = ctx.enter_context(tc.tile_pool(name="xnpool", bufs=2))
        gpool = ctx.enter_context(tc.tile_pool(name="gpool", bufs=2))
        qkpool = ctx.enter_context(tc.tile_pool(name="qkpool", bufs=2))
        vpool = ctx.enter_context(tc.tile_pool(name="vpool", bufs=2))
        ptpool = ctx.enter_context(tc.tile_pool(name="ptpool", bufs=8))
        cspool = ctx.enter_context(tc.tile_pool(name="cspool", bufs=2))
        rcpool = ctx.enter_context(tc.tile_pool(name="rcpool", bufs=2))
        opool = ctx.enter_context(tc.tile_pool(name="opool", bufs=3))
        outpool = ctx.enter_context(tc.tile_pool(name="outpool", bufs=3))
        psA = ctx.enter_context(tc.tile_pool(name="psA", bufs=6, space="PSUM"))
        psO = ctx.enter_context(tc.tile_pool(name="psO", bufs=2, space="PSUM"))

        # ---- constants (batched DMAs; x of sample 0 is issued first
        # because the groupnorm chain is the startup critical path) ----
        x0 = xpool.tile([P, CT, HW], F32, name="x_0", tag="x")
        for hh in range(NH):
            nc.sync.dma_start(out=x0[:, 0, _hs(hh)], in_=x_d[0, 0][:, _hs(hh)])
        nc.sync.dma_start(out=x0[:, 1], in_=x_d[0, 1])
        sm = consts.tile([P, 6 + CT * G], F32, name="sm_sb", tag="sm_sb")
        nc.sync.dma_start(out=sm, in_=sm_d[:])
        bcmask = consts.tile([G, CT * P], F32, name="bcmask_sb", tag="bcmask_sb")
        nc.sync.dma_start(out=bcmask, in_=bcmask_d[:])
        wall = consts.tile([P, CT, WW], MM_DT, name="wall_sb", tag="wall_sb")
        for ct in range(CT):
            nc.sync.dma_start(out=wall[:, ct], in_=wall_d[ct])
        bp = sm[:, 0:CT]
        gnw = sm[:, CT:2 * CT]
        gnb = sm[:, 2 * CT:3 * CT]
        ones32 = consts.tile([P, P], F32, name="ones32_sb", tag="ones32_sb")
        nc.vector.memset(ones32, 1.0)
        ones = consts.tile([P, P], MM_DT, name="ones_sb", tag="ones_sb")
        nc.vector.tensor_copy(ones, ones32)
        epst = consts.tile([P, 1], F32, name="eps_sb", tag="eps_sb")
        nc.vector.memset(epst, EPS)
        # warm up the PE clock (HAM) during the startup groupnorm chain so
        # the first real matmuls run at full rate
        dmy32 = consts.tile([P, FD], F32, name="dmy32_sb", tag="dmy32_sb")
        nc.vector.memset(dmy32, 1.0)
        dmy = consts.tile([P, FD], MM_DT, name="dmy_sb", tag="dmy_sb")
        nc.vector.tensor_copy(dmy, dmy32)
        wps = psA.tile([P, FD], F32, name="warm_ps", tag="ps")
        for i in range(12):
            nc.tensor.matmul(wps, lhsT=ones, rhs=dmy, start=True, stop=True,
                             skip_group_check=True)

        def emit_load(s):
            if s == 0:
                return x0
            xs = xpool.tile([P, CT, HW], F32, name=f"x_{s}", tag="x")
            for ct in range(CT):
                nc.sync.dma_start(out=xs[:, ct], in_=x_d[s, ct])
            return xs

        def emit_gn_stats1(s, xs):
            # per-channel stats (DVE only)
            st6 = gpool.tile([P, CT, 2, 6], F32, name=f"st6_{s}", tag="st6")
            mv = gpool.tile([P, CT, 2], F32, name=f"mv_{s}", tag="mv")
            ms = gpool.tile([P, CT, 2], F32, name=f"ms_{s}", tag="ms")
            for ct in range(CT):
                for h in range(2):
                    nc.vector.bn_stats(out=st6[:, ct, h], in_=xs[:, ct, _hs(h)])
                nc.vector.bn_aggr(out=mv[:, ct], in_=st6[:, ct])
                # ms = [mean, E[x^2]] per channel
                nc.vector.tensor_copy(ms[:, ct, 0:1], mv[:, ct, 0:1])
                nc.vector.tensor_tensor(
                    out=ms[:, ct, 1:2], in0=mv[:, ct, 0:1], in1=mv[:, ct, 0:1],
                    op=AL.mult)
                nc.vector.tensor_tensor(
                    out=ms[:, ct, 1:2], in0=ms[:, ct, 1:2], in1=mv[:, ct, 1:2],
                    op=AL.add)
            return ms

        def emit_gn_stats2(s, ms):
            # group aggregate (tiny PE matmul) + rstd chain
            gps = psA.tile([G, 2], F32, name=f"gps_{s}", tag="ps")
            for ct in range(CT):
                nc.tensor.matmul(gps,
                                 lhsT=sm[:, 3 * CT + ct * G:3 * CT + (ct + 1) * G],
                                 rhs=ms[:, ct],
                                 start=(ct == 0), stop=(ct == CT - 1))
            # group mean/rstd (PSUM stats go to SBUF first: the verifier
            # allows at most one PSUM input per DVE op)
            graw = gpool.tile([G, 2], F32, name=f"graw_{s}", tag="graw")
            gtmp = gpool.tile([G, 2], F32, name=f"gtmp_{s}", tag="gtmp")
            grs = gpool.tile([G, 2], F32, name=f"grs_{s}", tag="grs")
            nc.vector.tensor_copy(graw, gps)
            nc.vector.tensor_tensor(out=gtmp[:, 0:1], in0=graw[:, 0:1],
                                    in1=graw[:, 0:1], op=AL.mult)
            nc.vector.tensor_tensor(out=gtmp[:, 1:2], in0=graw[:, 1:2],
                                    in1=gtmp[:, 0:1], op=AL.subtract)
            # rstd = exp(-0.5*ln(var+eps)): ln and exp live in the same ACT
            # table set as the softmax exp, so no table reloads ever happen
            nc.scalar.activation(out=gtmp[:, 0:1], in_=gtmp[:, 1:2],
                                 func=AF.Ln, bias=epst[:G])
            nc.scalar.activation(out=grs[:, 1:2], in_=gtmp[:, 0:1],
                                 func=AF.Exp, scale=-0.5)
            nc.vector.tensor_copy(grs[:, 0:1], graw[:, 0:1])
            return grs

        def emit_gn_affine(s, grs, xs):
            # broadcast per-group [mean, rstd] back to channels, build affine
            AB = gpool.tile([P, CT, 2], F32, name=f"AB_{s}", tag="AB")
            xn = xnpool.tile([P, CT, HW], MM_DT, name=f"xn_{s}", tag="xn")
            for ct in range(CT):
                bc = psA.tile([P, 2], F32, name=f"bc_{s}_{ct}", tag="ps")
                nc.tensor.matmul(bc, lhsT=bcmask[:, ct * P:(ct + 1) * P],
                                 rhs=grs, start=True, stop=True)
                nc.vector.tensor_tensor(out=AB[:, ct, 0:1], in0=bc[:, 1:2],
                                        in1=gnw[:, ct:ct + 1], op=AL.mult)
                nc.vector.tensor_tensor(out=AB[:, ct, 1:2], in0=bc[:, 0:1],
                                        in1=AB[:, ct, 0:1], op=AL.mult)
                nc.vector.tensor_tensor(out=AB[:, ct, 1:2], in0=gnb[:, ct:ct + 1],
                                        in1=AB[:, ct, 1:2], op=AL.subtract)
                for hh in range(NH):
                    nc.vector.tensor_scalar(
                        out=xn[:, ct, _hs(hh)], in0=xs[:, ct, _hs(hh)],
                        scalar1=AB[:, ct, 0:1], scalar2=AB[:, ct, 1:2],
                        op0=AL.mult, op1=AL.add)
            return xn

        def emit_u(s, xn):
            # u = (Wk^T Wq) @ xn : the fused q.k operand.  Softmax over k is
            # invariant to per-q-column constants, so only the k-side bias
            # survives; it rides in column C of the v matmul.
            u = qkpool.tile([P, CT, HW], MM_DT, name=f"u_{s}", tag="u")
            for h in range(NH):
                for m in range(CT):
                    ps = psA.tile([P, FD], F32, name=f"u_ps_{s}_{m}_{h}", tag="ps")
                    for ct in range(CT):
                        nc.tensor.matmul(
                            ps,
                            lhsT=wall[:, ct, _ms(m)],
                            rhs=xn[:, ct, _hs(h)],
                            start=(ct == 0), stop=(ct == CT - 1))
                    nc.scalar.copy(out=u[:, m, _hs(h)], in_=ps)
            return u

        def emit_v_tile(s, xn, vT, mt):
            # one spatial tile of v, transposed: [spatial, c] (no bias:
            # v_bias is folded into the proj bias on host)
            ps = psA.tile([P, FD], F32, name=f"v_ps_{s}_{mt}", tag="ps")
            for ct in range(CT):
                nc.tensor.matmul(
                    ps[:, :C + 4],
                    lhsT=xn[:, ct, _ms(mt)],
                    rhs=wall[:, ct, C:2 * C + 4],
                    start=(ct == 0), stop=(ct == CT - 1))
            nc.scalar.copy(out=vT[:, mt], in_=ps[:, :C + 1])

        def emit_attn_half(s, h, xs, xn, u, vT):
            last_h = (s == NS - 1 and h == NH - 1)
            xsb = None
            if last_h:
                # pre-biased residual: shortens the fully-exposed final tail
                xsb = outpool.tile([P, CT, FD], F32, name="xsb", tag="xsb")
                for m in range(CT):
                    nc.vector.tensor_scalar_add(xsb[:, m], xs[:, m, _hs(h)],
                                                bp[:, m:m + 1])
            po = [psO.tile([P, FD], F32, name=f"po_{s}_{h}_{m}", tag="po")
                  for m in range(CT)]
            jit_v = (h == 0)
            csa = cspool.tile([P, FD], F32, name=f"csa_{s}_{h}", tag="csa")
            csb = cspool.tile([P, FD], F32, name=f"csb_{s}_{h}", tag="csb")
            cs = cspool.tile([P, FD], MM_DT, name=f"cs_{s}_{h}", tag="cs")
            for kt in range(NT):
                if jit_v:
                    emit_v_tile(s, xn, vT, kt)
                pt = ptpool.tile([P, FD], MM_DT, name=f"pt_{s}_{h}_{kt}",
                                 tag="pt")
                ps = psA.tile([P, FD], F32, name=f"st_ps_{s}_{h}_{kt}",
                              tag="ps")
                for ct in range(CT):
                    nc.tensor.matmul(
                        ps,
                        lhsT=u[:, ct, _ms(kt)],
                        rhs=xn[:, ct, _hs(h)],
                        start=(ct == 0), stop=(ct == CT - 1))
                nc.scalar.activation(out=pt, in_=ps, func=AF.Exp,
                                     scale=SCALE,
                                     bias=vT[:, kt, C:C + 1].bitcast(F32))
                # colsum partials: DVE takes most k-tiles, GPSIMD some;
                # partials merge at kt==6 and the last tile joins via PSUM
                # accumulation in the ones-matmul, so the post-loop
                # denominator chain is one matmul + reciprocal
                last_tail = (s == NS - 1 and h == NH - 1)
                for m in range(CT):
                    nc.tensor.matmul(
                        po[m],
                        lhsT=vT[:, kt, _ms(m)],
                        rhs=pt,
                        start=(kt == 0), stop=(kt == NT - 1))
                if kt == 0:
                    nc.vector.tensor_copy(csa, pt)
                elif kt == 1 and not last_tail:
                    nc.gpsimd.tensor_copy(csb, pt)
                elif kt == 3 and not last_tail:
                    nc.gpsimd.tensor_tensor(out=csb, in0=csb, in1=pt,
                                            op=AL.add)
                elif kt == 5 and not last_tail:
                    nc.gpsimd.tensor_tensor(out=csb, in0=csb, in1=pt,
                                            op=AL.add)
                elif kt < 7:
                    nc.vector.tensor_tensor(out=csa, in0=csa, in1=pt,
                                            op=AL.add)
                if kt == 6:
                    if last_tail:
                        nc.vector.tensor_copy(cs, csa)
                    else:
                        nc.vector.tensor_tensor(out=cs, in0=csa, in1=csb,
                                                op=AL.add)
                elif kt == 7:
                    pt7 = pt
            # denominator: partition-sum + broadcast via all-ones matmul;
            # k-tiles 0..6 arrive pre-merged in cs, tile 7 accumulates in PSUM
            if last_h:
                psc = psA.tile([P, FD], F32, name=f"cs_ps_{s}_{h}", tag="ps")
            else:
                psc = psO.tile([P, FD], F32, name=f"cs_ps_{s}_{h}", tag="po")
            nc.tensor.matmul(psc, lhsT=ones, rhs=cs, start=True, stop=False)
            nc.tensor.matmul(psc, lhsT=ones, rhs=pt7, start=False, stop=True)
            recip = rcpool.tile([P, FD], F32, name=f"recip_{s}_{h}",
                                tag="recip")
            nc.vector.reciprocal(out=recip, in_=psc)
            # proj consumes UNNORMALIZED attention output (normalization
            # commutes with the channel contraction); the 1/colsum factor is
            # applied post-proj so proj doesn't wait on the denominator
            osb = opool.tile([P, CT, FD], MM_DT, name=f"osb_{s}_{h}",
                             tag="osb")
            nc.scalar.copy(out=osb[:, 0], in_=po[0])
            nc.scalar.copy(out=osb[:, 1], in_=po[1])
            outp = outpool.tile([P, CT, FD], F32, name=f"outp_{s}_{h}",
                                tag="outp")
            for m in range(CT):
                ps = psO.tile([P, FD], F32, name=f"p_ps_{s}_{h}_{m}",
                              tag="po")
                for ct in range(CT):
                    nc.tensor.matmul(
                        ps,
                        lhsT=wall[:, ct, 2 * C + 4 + m * P:2 * C + 4 + (m + 1) * P],
                        rhs=osb[:, ct],
                        start=(ct == 0), stop=(ct == CT - 1))
                nc.vector.tensor_tensor(out=outp[:, m], in0=ps,
                                        in1=recip, op=AL.mult)
                if last_h:
                    # final tail is fully exposed: fastest engine + early DMA
                    nc.vector.tensor_tensor(out=outp[:, m], in0=outp[:, m],
                                            in1=xsb[:, m], op=AL.add)
                    nc.sync.dma_start(out=out_d[s, m][:, _hs(h)],
                                      in_=outp[:, m])
                else:
                    nc.vector.tensor_scalar_add(outp[:, m], outp[:, m],
                                                bp[:, m:m + 1])
                    nc.gpsimd.tensor_tensor(out=outp[:, m], in0=outp[:, m],
                                            in1=xs[:, m, _hs(h)], op=AL.add)
            if not last_h:
                nc.sync.dma_start(
                    out=out_d[s][:, :, _hs(h)].rearrange("ct p f -> p ct f"),
                    in_=outp)

        # ---- software-pipelined emission: the next sample's loads and
        # groupnorm stages are traced at points where their inputs are
        # already available, so the in-order engine streams never
        # head-of-line-block on a cross-engine chain ----
        xs_l = [None] * NS
        xs_l[0] = emit_load(0)
        ms0 = emit_gn_stats1(0, xs_l[0])
        grs0 = emit_gn_stats2(0, ms0)
        xn_cur = emit_gn_affine(0, grs0, xs_l[0])
        if NS > 1:
            xs_l[1] = emit_load(1)
        for s in range(NS):
            u = emit_u(s, xn_cur)
            vT = vpool.tile([P, NT, C + 1], MM_DT, name=f"vT_{s}", tag="vT")
            if s + 2 < NS:
                xs_l[s + 2] = emit_load(s + 2)
            ms_nxt = emit_gn_stats1(s + 1, xs_l[s + 1]) if s + 1 < NS else None
            emit_attn_half(s, 0, xs_l[s], xn_cur, u, vT)
            xn_nxt = None
            if s + 1 < NS:
                grs_nxt = emit_gn_stats2(s + 1, ms_nxt)
                xn_nxt = emit_gn_affine(s + 1, grs_nxt, xs_l[s + 1])
            emit_attn_half(s, 1, xs_l[s], xn_cur, u, vT)
            xn_cur = xn_nxt

    # f32r matmuls are self-loading (no separate LDWEIGHTS to absorb waits)
    # and the S3 LW struct only has one wait slot; split excess waits onto
    # InstEventSemaphore instructions.
    import bass_rust
    bass_rust.generate_event_semaphores(nc)
    return nc


def _get_nc():
    if "nc" not in _nc_cache:
        _nc_cache["nc"] = _build_nc()
    return _nc_cache["nc"]


def _prep_consts(gn_w, gn_b, qkv_w, qkv_b, proj_w, proj_b):
    f = np.float32
    c = np.ascontiguousarray
    Wq = qkv_w[:C].astype(np.float64)
    Wk = qkv_w[C:2 * C].astype(np.float64)
    bq = qkv_b[:C].astype(np.float64)
    scale = C ** -0.5
    wu = (Wk.T @ Wq).astype(f)
    # v weights plus the surviving k-side logit bias (pre-scaled) as col C
    wve = np.concatenate(
        [qkv_w[2 * C:].T.astype(np.float64), (scale * (Wk.T @ bq))[:, None],
         np.zeros((C, 3))],
        axis=1).astype(f)
    wpt = proj_w.T.astype(f)
    wall = c(np.concatenate([wu, wve, wpt], axis=1).reshape(CT, P, 3 * C + 4))
    # softmax rows sum to 1, so the v bias contributes proj_w @ v_bias to
    # every output position; fold it into the proj bias on the host.
    bp_eff = (proj_b.astype(np.float64)
              + proj_w.astype(np.float64) @ qkv_b[2 * C:].astype(np.float64))
    bp = bp_eff.astype(f).reshape(CT, P).T
    gnw = gn_w.reshape(CT, P).T.astype(f)
    gnb = gn_b.reshape(CT, P).T.astype(f)
    cidx = np.arange(C)
    grp = cidx // (C // G)
    gmask = np.zeros((CT, P, G), f)
    gmask[cidx // P, cidx % P, grp] = 1.0 / (C // G)
    sm = c(np.concatenate(
        [bp, gnw, gnb, gmask.transpose(1, 0, 2).reshape(P, CT * G)], axis=1))
    bcmask = np.zeros((G, CT * P), f)
    bcmask[grp, cidx] = 1.0
    return dict(wall=wall, sm=sm, bcmask=bcmask)


def kernel(x, gn_w, gn_b, qkv_w, qkv_b, proj_w, proj_b):
    global last_results
    x = np.ascontiguousarray(np.asarray(x, dtype=np.float32))
    consts = _prep_consts(
        np.asarray(gn_w, np.float32), np.asarray(gn_b, np.float32),
        np.asarray(qkv_w, np.float32), np.asarray(qkv_b, np.float32),
        np.asarray(proj_w, np.float32), np.asarray(proj_b, np.float32))
    nc = _get_nc()
    xr = x.reshape(NCORES, NS, CT, P, HW)
    in_maps = [dict(x=np.ascontiguousarray(xr[i]), **consts)
               for i in range(NCORES)]
    trace = bool(int(os.environ.get("ATTN_TRACE", "0")))
    last_results = run_bass_kernel_spmd(
        nc, in_maps, core_ids=list(range(NCORES)), trace=trace)
    out = np.stack([r["out"] for r in last_results.results])
    return out.reshape(B, C, HIMG, WIMG)

